# revision 5
# baseline (speedup 1.0000x reference)
"""Trainium2 (8 NeuronCores) kernel for a GPT-2 style causal attention block.

Reference math (per batch b):
    qkv = x @ W_attn + b_attn            # [T, 3E]
    q,k,v split -> heads H=16, D=64
    att = softmax(mask(q k^T / sqrt(D))) # causal mask
    y   = (att @ v) @ W_proj + b_proj    # [T, E]

Sharding (8 cores, no collectives):
    core c = (batch b = c//2, head-group hg = c%2 of 8 heads).
    Each core computes a PARTIAL y[b] = O_local @ W_proj[rows of its heads].
    Host sums the two partials per batch and adds b_proj (exact, commutes).

Device kernel per core (all bf16 matmuls, fp32 PSUM accumulate):
    phase 1: Q^T, K^T (feats on partitions) and V (rows on partitions) via
             matmuls from host-fed x^T and W shards.  1/sqrt(D) is folded
             into the Q columns of W on the host (exact: /8 is a pow2).
    phase 2: per (head, q-chunk of 512): S^T tiles [128 k, 512 q] on PE,
             exp on ACT (no max-subtraction needed: scores are O(1) by
             construction), causal masking by memset + multiply with a
             128x128 triangular tile, O'^T accumulation with V' that has a
             ones-column appended -> row 64 of O' is the softmax denominator.
             Normalization is fused into the PSUM->SBUF copy (broadcast the
             denominator row via DMA, reciprocal, multiply).
    phase 3: y_partial = O @ W_proj_shard, PSUM -> SBUF -> DRAM (f32).
"""

import os
import numpy as np
import ml_dtypes

B, T, E, H = 4, 2048, 1024, 16
D = E // H            # 64
NCORES = 8
HL = H // 2           # local heads per core
DL = HL * D           # 512 local attention feats
QC = 512              # q-chunk width
NQC = T // QC         # 4
NKT = T // 128        # 16 k-tiles
P = 128

BF16 = ml_dtypes.bfloat16

_graph_cache = {}
LAST_RESULT = None    # BassKernelResults of the most recent run (for test.py)


def _build(causal: bool, with_bias: bool):
    import concourse.bass as bass  # noqa: F401
    import concourse.tile as tile
    from concourse import bacc, mybir
    from concourse.masks import make_upper_triangular

    bf16 = mybir.dt.bfloat16
    f32 = mybir.dt.float32
    Exp = mybir.ActivationFunctionType.Exp

    KIN = 1152 if with_bias else 1024   # qkv contraction (pad bias row to a full tile)
    NKIN = KIN // P

    nc = bacc.Bacc("TRN2", target_bir_lowering=False, debug=False,
                   num_devices=NCORES)
    xT = nc.declare_dram_parameter("xT", [KIN, T], bf16, isOutput=False)
    wqkv = nc.declare_dram_parameter("wqkv", [KIN, 3 * DL], bf16, isOutput=False)
    wproj = nc.declare_dram_parameter("wproj", [DL, E], bf16, isOutput=False)
    if not causal:
        maskT = nc.declare_dram_parameter("maskT", [T, T], bf16, isOutput=False)
    out = nc.declare_dram_parameter("out", [T, E], f32, isOutput=True)

    with tile.TileContext(nc) as tc, \
         tc.tile_pool(name="persist", bufs=1) as persist:
        # ---- persistent SBUF tensors ----
        xT_sb = persist.tile([P, NKIN, T], bf16, tag="xT_sb", name="xT_sb")
        wq_sb = persist.tile([P, NKIN, 3 * DL], bf16, tag="wq_sb", name="wq_sb")
        wp_sb = persist.tile([P, 4, E], bf16, tag="wp_sb", name="wp_sb")
        qT_sb = persist.tile([P, 4, T], bf16, tag="qT_sb", name="qT_sb")
        kT_sb = persist.tile([P, 4, T], bf16, tag="kT_sb", name="kT_sb")
        vP_sb = persist.tile([P, NKT, HL, D + 1], bf16, tag="vP_sb", name="vP_sb")
        oT_sb = persist.tile([P, 4, T], bf16, tag="oT_sb", name="oT_sb")
        band = persist.tile([P, P], bf16, tag="band", name="band")

        for kt in range(NKIN):
            nc.sync.dma_start(out=xT_sb[:, kt, :], in_=xT[kt * P:(kt + 1) * P, :])
            nc.sync.dma_start(out=wq_sb[:, kt, :], in_=wqkv[kt * P:(kt + 1) * P, :])
        for g in range(4):
            nc.sync.dma_start(out=wp_sb[:, g, :], in_=wproj[g * P:(g + 1) * P, :])
        if causal:
            # band[kp, qf] = 1.0 where kp <= qf else 0  (keep k <= q)
            make_upper_triangular(nc, band[:, :], val=1.0, diag=True)
        nc.vector.memset(vP_sb[:, :, :, D:D + 1], 1.0)

        with (
            tc.tile_pool(name="psA", bufs=2, space="PSUM") as psA,
            tc.tile_pool(name="psS", bufs=2, space="PSUM") as psS,
            tc.tile_pool(name="psO", bufs=2, space="PSUM") as psO,
            tc.tile_pool(name="sbw", bufs=3) as sbw,
            tc.tile_pool(name="sbm", bufs=2) as sbm,
            tc.tile_pool(name="drp", bufs=2, space="DRAM") as drp,
        ):
            # ---- phase 1a: V = x @ Wv  (rows on partitions) ----
            for rt in range(NKT):
                ps_v = psA.tile([P, DL], f32, tag="mm512", name="ps_v")
                for kt in range(NKIN):
                    nc.tensor.matmul(
                        ps_v[:],
                        lhsT=xT_sb[:, kt, rt * P:(rt + 1) * P],
                        rhs=wq_sb[:, kt, 2 * DL:3 * DL],
                        start=(kt == 0), stop=(kt == NKIN - 1))
                nc.vector.tensor_copy(
                    vP_sb[:, rt, :, 0:D],
                    ps_v[:].rearrange("p (h d) -> p h d", h=HL))

            for g in range(4):
                # ---- phase 1b: Q^T, K^T for head-pair g ----
                for rc in range(NQC):
                    ps_q = psA.tile([P, QC], f32, tag="mm512", name="ps_q")
                    for kt in range(NKIN):
                        nc.tensor.matmul(
                            ps_q[:],
                            lhsT=wq_sb[:, kt, g * P:(g + 1) * P],
                            rhs=xT_sb[:, kt, rc * QC:(rc + 1) * QC],
                            start=(kt == 0), stop=(kt == NKIN - 1))
                    nc.vector.tensor_copy(qT_sb[:, g, rc * QC:(rc + 1) * QC], ps_q[:])
                    ps_k = psA.tile([P, QC], f32, tag="mm512", name="ps_k")
                    for kt in range(NKIN):
                        nc.tensor.matmul(
                            ps_k[:],
                            lhsT=wq_sb[:, kt, DL + g * P:DL + (g + 1) * P],
                            rhs=xT_sb[:, kt, rc * QC:(rc + 1) * QC],
                            start=(kt == 0), stop=(kt == NKIN - 1))
                    nc.vector.tensor_copy(kT_sb[:, g, rc * QC:(rc + 1) * QC], ps_k[:])

                # ---- phase 2: attention for heads 2g, 2g+1 ----
                for j in range(2):
                    hl = 2 * g + j
                    for qc in range(NQC):
                        nkt = 4 * (qc + 1) if causal else NKT
                        ps_o = psO.tile([P, QC], f32, name="ps_o")
                        for kt2 in range(nkt // 2):
                            ps_s = psS.tile([P, 2 * QC], f32, name="ps_s")
                            for t2 in range(2):
                                kt = 2 * kt2 + t2
                                nc.tensor.matmul(
                                    ps_s[:, t2 * QC:(t2 + 1) * QC],
                                    lhsT=kT_sb[j * D:(j + 1) * D, g, kt * P:(kt + 1) * P],
                                    rhs=qT_sb[j * D:(j + 1) * D, g, qc * QC:(qc + 1) * QC],
                                    start=True, stop=True)
                            pT = sbw.tile([P, 2 * QC], bf16, tag="pT", name="pT")
                            nc.scalar.activation(out=pT[:], in_=ps_s[:], func=Exp)
                            if causal:
                                for t2 in range(2):
                                    kt = 2 * kt2 + t2
                                    if kt >= 4 * qc:  # diagonal-band k-tile
                                        s = kt * P - qc * QC
                                        if s > 0:
                                            nc.vector.memset(pT[:, t2 * QC:t2 * QC + s], 0.0)
                                        nc.vector.tensor_mul(
                                            pT[:, t2 * QC + s:t2 * QC + s + P],
                                            pT[:, t2 * QC + s:t2 * QC + s + P],
                                            band[:, :])
                            else:
                                msk = sbm.tile([P, 2 * QC], bf16, tag="msk", name="msk")
                                for t2 in range(2):
                                    kt = 2 * kt2 + t2
                                    nc.sync.dma_start(
                                        out=msk[:, t2 * QC:(t2 + 1) * QC],
                                        in_=maskT[kt * P:(kt + 1) * P, qc * QC:(qc + 1) * QC])
                                nc.vector.tensor_mul(pT[:], pT[:], msk[:])
                            for t2 in range(2):
                                kt = 2 * kt2 + t2
                                nc.tensor.matmul(
                                    ps_o[0:D + 1, :],
                                    lhsT=vP_sb[:, kt, hl, :],
                                    rhs=pT[:, t2 * QC:(t2 + 1) * QC],
                                    start=(kt == 0), stop=(kt == nkt - 1))
                        # normalize: O[d, q] / rowsum[q], rowsum = ps_o row D
                        rrow = sbm.tile([1, QC], f32, tag="rrow", name="rrow")
                        nc.vector.tensor_copy(rrow[:], ps_o[D:D + 1, :])
                        rdr = drp.tile([1, QC], f32, tag="rdr", name="rdr")
                        nc.sync.dma_start(out=rdr[:], in_=rrow[:])
                        rb = sbm.tile([D, QC], f32, tag="rb", name="rb")
                        nc.sync.dma_start(out=rb[:], in_=rdr[:].to_broadcast((D, QC)))
                        nc.vector.reciprocal(rb[:], rb[:])
                        nc.vector.tensor_mul(
                            oT_sb[j * D:(j + 1) * D, g, qc * QC:(qc + 1) * QC],
                            ps_o[0:D, :], rb[:])

            # ---- phase 3: y_partial = O @ W_proj_shard ----
            for rt in range(NKT):
                for nb in range(2):
                    ps_y = psA.tile([P, 512], f32, tag="mm512", name="ps_y")
                    for g in range(4):
                        nc.tensor.matmul(
                            ps_y[:],
                            lhsT=oT_sb[:, g, rt * P:(rt + 1) * P],
                            rhs=wp_sb[:, g, nb * 512:(nb + 1) * 512],
                            start=(g == 0), stop=(g == 3))
                    y_sb = sbw.tile([P, 512], f32, tag="y_sb", name="y_sb")
                    nc.vector.tensor_copy(y_sb[:], ps_y[:])
                    nc.sync.dma_start(
                        out=out[rt * P:(rt + 1) * P, nb * 512:(nb + 1) * 512],
                        in_=y_sb[:])

    nc.compile()
    return nc


def _get_graph(causal: bool, with_bias: bool):
    key = (causal, with_bias)
    if key not in _graph_cache:
        _graph_cache[key] = _build(causal, with_bias)
    return _graph_cache[key]


def make_in_maps(x, mask, W_attn, b_attn, W_proj, b_proj, causal, with_bias):
    """Host-side sharding: per-core input dict (bf16)."""
    in_maps = []
    maskT_bf = None
    if not causal:
        m = np.asarray(mask).reshape(T, T)
        maskT_bf = np.ascontiguousarray(m.T).astype(BF16)
    for c in range(NCORES):
        b, hg = c // 2, c % 2
        lo, hi = hg * DL, (hg + 1) * DL
        Wq = W_attn[:, lo:hi] * np.float32(0.125)
        Wk = W_attn[:, E + lo:E + hi]
        Wv = W_attn[:, 2 * E + lo:2 * E + hi]
        wqkv = np.concatenate([Wq, Wk, Wv], axis=1).astype(np.float32)
        xt = np.ascontiguousarray(x[b].T).astype(np.float32)
        if with_bias:
            brow = np.concatenate([
                b_attn[lo:hi] * np.float32(0.125),
                b_attn[E + lo:E + hi],
                b_attn[2 * E + lo:2 * E + hi]]).astype(np.float32)
            wqkv = np.concatenate(
                [wqkv, brow[None, :], np.zeros((P - 1, 3 * DL), np.float32)], axis=0)
            xt = np.concatenate(
                [xt, np.ones((1, T), np.float32), np.zeros((P - 1, T), np.float32)],
                axis=0)
        im = {
            "xT": np.ascontiguousarray(xt).astype(BF16),
            "wqkv": np.ascontiguousarray(wqkv).astype(BF16),
            "wproj": np.ascontiguousarray(W_proj[lo:hi, :]).astype(BF16),
        }
        if not causal:
            im["maskT"] = maskT_bf
        in_maps.append(im)
    return in_maps


def expected_partial(x, mask, W_attn, b_attn, W_proj, core):
    """Numpy reference for ONE core's partial output (for sim testing)."""
    b, hg = core // 2, core % 2
    lo, hi = hg * DL, (hg + 1) * DL
    q = x[b] @ W_attn[:, lo:hi] + b_attn[lo:hi]
    k = x[b] @ W_attn[:, E + lo:E + hi] + b_attn[E + lo:E + hi]
    v = x[b] @ W_attn[:, 2 * E + lo:2 * E + hi] + b_attn[2 * E + lo:2 * E + hi]
    q = q.reshape(T, HL, D)
    k = k.reshape(T, HL, D)
    v = v.reshape(T, HL, D)
    att = np.einsum('qhd,khd->hqk', q, k) / np.sqrt(D)
    m = np.asarray(mask).reshape(T, T)
    att = np.where(m[None] == 0, np.float32(-1e20), att)
    att = att - att.max(axis=-1, keepdims=True)
    att = np.exp(att)
    att = att / att.sum(axis=-1, keepdims=True)
    o = np.einsum('hqk,khd->qhd', att, v).reshape(T, DL)
    return o @ W_proj[lo:hi, :]


def kernel(x, mask, W_attn, b_attn, W_proj, b_proj):
    global LAST_RESULT
    from concourse.bass_utils import run_bass_kernel_spmd

    x = np.asarray(x, dtype=np.float32)
    W_attn = np.asarray(W_attn, dtype=np.float32)
    b_attn = np.asarray(b_attn, dtype=np.float32)
    W_proj = np.asarray(W_proj, dtype=np.float32)
    b_proj = np.asarray(b_proj, dtype=np.float32)

    mask2d = np.asarray(mask).reshape(T, T)
    causal = bool(np.array_equal(mask2d != 0, np.tril(np.ones((T, T), bool))))
    if not causal and not (mask2d != 0).any(axis=1).all():
        # A fully-masked query row: reference softmax degenerates to uniform
        # attention; not representable in the 0/1-multiply fast path.  This
        # cannot occur for the causal mask; fall back to exact host math.
        y = np.stack([
            sum(expected_partial(x, mask, W_attn, b_attn, W_proj, 2 * b + hg)
                for hg in range(2))
            for b in range(B)]).astype(np.float32)
        return y + b_proj
    with_bias = bool(np.any(b_attn))

    nc = _get_graph(causal, with_bias)
    in_maps = make_in_maps(x, mask, W_attn, b_attn, W_proj, b_proj,
                           causal, with_bias)
    trace = bool(int(os.environ.get("CK_TRACE", "0")))
    res = run_bass_kernel_spmd(nc, in_maps, core_ids=list(range(NCORES)),
                               trace=trace)
    LAST_RESULT = res
    y = np.empty((B, T, E), np.float32)
    for b in range(B):
        y[b] = res.results[2 * b]["out"].astype(np.float32) \
             + res.results[2 * b + 1]["out"].astype(np.float32)
    return y + b_proj


# revision 8
# speedup vs baseline: 1.3080x; 1.3080x over previous
"""Trainium2 (8 NeuronCores) kernel for a GPT-2 style causal attention block.

Reference math (per batch b):
    qkv = x @ W_attn + b_attn            # [T, 3E]
    q,k,v split -> heads H=16, D=64
    att = softmax(mask(q k^T / sqrt(D))) # causal mask
    y   = (att @ v) @ W_proj + b_proj    # [T, E]

Sharding (8 cores, no collectives):
    core c = (batch b = c//2, head-group hg = c%2 of 8 heads).
    Each core computes a PARTIAL y[b] = O_local @ W_proj[rows of its heads].
    Host sums the two partials per batch and adds b_proj (exact, commutes).

Device kernel per core (all bf16 matmuls, fp32 PSUM accumulate):
    phase 1: Q^T, K^T (feats on partitions) and V (rows on partitions) via
             matmuls from host-fed x^T and W shards.  1/sqrt(D) is folded
             into the Q columns of W on the host (exact: /8 is a pow2).
    phase 2: per (head, q-chunk of 512): S^T tiles [128 k, 512 q] on PE,
             exp on ACT (no max-subtraction needed: scores are O(1) by
             construction), causal masking by memset + multiply with a
             128x128 triangular tile, O'^T accumulation with V' that has a
             ones-column appended -> row 64 of O' is the softmax denominator.
             Normalization is fused into the PSUM->SBUF copy (broadcast the
             denominator row via DMA, reciprocal, multiply).
    phase 3: y_partial = O @ W_proj_shard, PSUM -> SBUF -> DRAM (f32).
"""

import os
import numpy as np
import ml_dtypes

B, T, E, H = 4, 2048, 1024, 16
D = E // H            # 64
NCORES = 8
HL = H // 2           # local heads per core
DL = HL * D           # 512 local attention feats
QC = 512              # q-chunk width
NQC = T // QC         # 4
NKT = T // 128        # 16 k-tiles
P = 128

BF16 = ml_dtypes.bfloat16

_graph_cache = {}
LAST_RESULT = None    # BassKernelResults of the most recent run (for test.py)


def _build(causal: bool, with_bias: bool):
    import concourse.bass as bass  # noqa: F401
    import concourse.tile as tile
    from concourse import bacc, mybir
    from concourse.masks import make_upper_triangular

    bf16 = mybir.dt.bfloat16
    f32 = mybir.dt.float32
    Exp = mybir.ActivationFunctionType.Exp

    KIN = 1152 if with_bias else 1024   # qkv contraction (pad bias row to a full tile)
    NKIN = KIN // P

    nc = bacc.Bacc("TRN2", target_bir_lowering=False, debug=False,
                   num_devices=NCORES)
    xT = nc.declare_dram_parameter("xT", [KIN, T], bf16, isOutput=False)
    wqkv = nc.declare_dram_parameter("wqkv", [KIN, 3 * DL], bf16, isOutput=False)
    wproj = nc.declare_dram_parameter("wproj", [DL, E], bf16, isOutput=False)
    if not causal:
        maskT = nc.declare_dram_parameter("maskT", [T, T], bf16, isOutput=False)
    out = nc.declare_dram_parameter("out", [T, E], f32, isOutput=True)

    with tile.TileContext(nc) as tc, \
         tc.tile_pool(name="persist", bufs=1) as persist:
        # ---- persistent SBUF tensors ----
        xT_sb = persist.tile([P, NKIN, T], bf16, tag="xT_sb", name="xT_sb")
        wq_sb = persist.tile([P, NKIN, 3 * DL], bf16, tag="wq_sb", name="wq_sb")
        wp_sb = persist.tile([P, 4, E], bf16, tag="wp_sb", name="wp_sb")
        qT_sb = persist.tile([P, 4, T], bf16, tag="qT_sb", name="qT_sb")
        kT_sb = persist.tile([P, 4, T], bf16, tag="kT_sb", name="kT_sb")
        vP_sb = persist.tile([P, NKT, HL, D + 1], bf16, tag="vP_sb", name="vP_sb")
        oT_sb = persist.tile([P, 4, T], bf16, tag="oT_sb", name="oT_sb")
        band = persist.tile([P, P], bf16, tag="band", name="band")

        for kt in range(NKIN):
            nc.sync.dma_start(out=xT_sb[:, kt, :], in_=xT[kt * P:(kt + 1) * P, :])
            nc.sync.dma_start(out=wq_sb[:, kt, :], in_=wqkv[kt * P:(kt + 1) * P, :])
        for g in range(4):
            nc.sync.dma_start(out=wp_sb[:, g, :], in_=wproj[g * P:(g + 1) * P, :])
        if causal:
            # band[kp, qf] = 1.0 where kp <= qf else 0  (keep k <= q)
            make_upper_triangular(nc, band[:, :], val=1.0, diag=True)
        nc.vector.memset(vP_sb[:, :, :, D:D + 1], 1.0)

        with (
            tc.tile_pool(name="psA", bufs=2, space="PSUM") as psA,
            tc.tile_pool(name="psS", bufs=2, space="PSUM") as psS,
            tc.tile_pool(name="psO", bufs=2, space="PSUM") as psO,
            tc.tile_pool(name="sbw", bufs=3) as sbw,
            tc.tile_pool(name="sbm", bufs=2) as sbm,
            tc.tile_pool(name="drp", bufs=2, space="DRAM") as drp,
        ):
            # ---- phase 1a: V = x @ Wv  (rows on partitions) ----
            for rt in range(NKT):
                ps_v = psA.tile([P, DL], f32, tag="mm512", name="ps_v")
                for kt in range(NKIN):
                    nc.tensor.matmul(
                        ps_v[:],
                        lhsT=xT_sb[:, kt, rt * P:(rt + 1) * P],
                        rhs=wq_sb[:, kt, 2 * DL:3 * DL],
                        start=(kt == 0), stop=(kt == NKIN - 1))
                nc.vector.tensor_copy(
                    vP_sb[:, rt, :, 0:D],
                    ps_v[:].rearrange("p (h d) -> p h d", h=HL))

            def emit_qk(g):
                # ---- phase 1b: Q^T, K^T for head-pair g ----
                for rc in range(NQC):
                    ps_q = psA.tile([P, QC], f32, tag="mm512", name="ps_q")
                    for kt in range(NKIN):
                        nc.tensor.matmul(
                            ps_q[:],
                            lhsT=wq_sb[:, kt, g * P:(g + 1) * P],
                            rhs=xT_sb[:, kt, rc * QC:(rc + 1) * QC],
                            start=(kt == 0), stop=(kt == NKIN - 1))
                    nc.vector.tensor_copy(qT_sb[:, g, rc * QC:(rc + 1) * QC], ps_q[:])
                    ps_k = psA.tile([P, QC], f32, tag="mm512", name="ps_k")
                    for kt in range(NKIN):
                        nc.tensor.matmul(
                            ps_k[:],
                            lhsT=wq_sb[:, kt, DL + g * P:DL + (g + 1) * P],
                            rhs=xT_sb[:, kt, rc * QC:(rc + 1) * QC],
                            start=(kt == 0), stop=(kt == NKIN - 1))
                    nc.vector.tensor_copy(kT_sb[:, g, rc * QC:(rc + 1) * QC], ps_k[:])

            def emit_attn(g):
                # ---- phase 2: attention for heads 2g, 2g+1, interleaved ----
                for qc in range(NQC):
                    nkt = 4 * (qc + 1) if causal else NKT
                    ps_o = [psO.tile([P, QC], f32, tag="ps_o", name=f"ps_o{j}") for j in range(2)]
                    for kt2 in range(nkt // 2):
                        pTs = []
                        for j in range(2):
                            ps_s = psS.tile([P, 2 * QC], f32, tag="ps_s", name=f"ps_s{j}")
                            for t2 in range(2):
                                kt = 2 * kt2 + t2
                                nc.tensor.matmul(
                                    ps_s[:, t2 * QC:(t2 + 1) * QC],
                                    lhsT=kT_sb[j * D:(j + 1) * D, g, kt * P:(kt + 1) * P],
                                    rhs=qT_sb[j * D:(j + 1) * D, g, qc * QC:(qc + 1) * QC],
                                    start=True, stop=True)
                            pT = sbw.tile([P, 2 * QC], bf16, tag=f"pT{j}", name=f"pT{j}")
                            pTs.append(pT)
                            if causal and 2 * kt2 >= 4 * qc and (s0 := 2 * kt2 * P - qc * QC) >= 2 * P:
                                # steep diagonal group: exp only live columns
                                nc.scalar.activation(out=pT[:, s0:QC], in_=ps_s[:, s0:QC], func=Exp)
                                nc.scalar.activation(out=pT[:, QC + s0 + P:], in_=ps_s[:, QC + s0 + P:], func=Exp)
                            else:
                                nc.scalar.activation(out=pT[:], in_=ps_s[:], func=Exp)
                            if causal:
                                for t2 in range(2):
                                    kt = 2 * kt2 + t2
                                    if kt >= 4 * qc:  # diagonal-band k-tile
                                        s = kt * P - qc * QC
                                        if s > 0:
                                            nc.vector.memset(pT[:, t2 * QC:t2 * QC + s], 0.0)
                                        nc.vector.tensor_mul(
                                            pT[:, t2 * QC + s:t2 * QC + s + P],
                                            pT[:, t2 * QC + s:t2 * QC + s + P],
                                            band[:, :])
                            else:
                                msk = sbm.tile([P, 2 * QC], bf16, tag="msk", name="msk")
                                for t2 in range(2):
                                    kt = 2 * kt2 + t2
                                    nc.sync.dma_start(
                                        out=msk[:, t2 * QC:(t2 + 1) * QC],
                                        in_=maskT[kt * P:(kt + 1) * P, qc * QC:(qc + 1) * QC])
                                nc.vector.tensor_mul(pT[:], pT[:], msk[:])
                        for j in range(2):
                            for t2 in range(2):
                                kt = 2 * kt2 + t2
                                nc.tensor.matmul(
                                    ps_o[j][0:D + 1, :],
                                    lhsT=vP_sb[:, kt, 2 * g + j, :],
                                    rhs=pTs[j][:, t2 * QC:(t2 + 1) * QC],
                                    start=(kt == 0), stop=(kt == nkt - 1))
                    for j in range(2):
                        # normalize: O[d, q] / rowsum[q], rowsum = ps_o row D
                        rrow = sbm.tile([1, QC], f32, tag="rrow", name="rrow")
                        nc.vector.tensor_copy(rrow[:], ps_o[j][D:D + 1, :])
                        rdr = drp.tile([1, QC], f32, tag="rdr", name="rdr")
                        nc.sync.dma_start(out=rdr[:], in_=rrow[:])
                        rb = sbm.tile([D, QC], f32, tag="rb", name="rb")
                        nc.sync.dma_start(out=rb[:], in_=rdr[:].to_broadcast((D, QC)))
                        nc.vector.reciprocal_approx_fast(out=rb[:], in_=rb[:])
                        nc.vector.tensor_mul(
                            oT_sb[j * D:(j + 1) * D, g, qc * QC:(qc + 1) * QC],
                            ps_o[j][0:D, :], rb[:])

            # software-pipeline the emission: QK of pair g+1 ahead of attn(g)
            emit_qk(0)
            for g in range(4):
                if g + 1 < 4:
                    emit_qk(g + 1)
                emit_attn(g)

            # ---- phase 3: y_partial = O @ W_proj_shard ----
            for rt in range(NKT):
                for nb in range(2):
                    ps_y = psA.tile([P, 512], f32, tag="mm512", name="ps_y")
                    for g in range(4):
                        nc.tensor.matmul(
                            ps_y[:],
                            lhsT=oT_sb[:, g, rt * P:(rt + 1) * P],
                            rhs=wp_sb[:, g, nb * 512:(nb + 1) * 512],
                            start=(g == 0), stop=(g == 3))
                    y_sb = sbw.tile([P, 512], f32, tag="y_sb", name="y_sb")
                    nc.vector.tensor_copy(y_sb[:], ps_y[:])
                    nc.sync.dma_start(
                        out=out[rt * P:(rt + 1) * P, nb * 512:(nb + 1) * 512],
                        in_=y_sb[:])

    nc.compile()
    return nc


def _get_graph(causal: bool, with_bias: bool):
    key = (causal, with_bias)
    if key not in _graph_cache:
        _graph_cache[key] = _build(causal, with_bias)
    return _graph_cache[key]


def make_in_maps(x, mask, W_attn, b_attn, W_proj, b_proj, causal, with_bias):
    """Host-side sharding: per-core input dict (bf16)."""
    in_maps = []
    maskT_bf = None
    if not causal:
        m = np.asarray(mask).reshape(T, T)
        maskT_bf = np.ascontiguousarray(m.T).astype(BF16)
    for c in range(NCORES):
        b, hg = c // 2, c % 2
        lo, hi = hg * DL, (hg + 1) * DL
        Wq = W_attn[:, lo:hi] * np.float32(0.125)
        Wk = W_attn[:, E + lo:E + hi]
        Wv = W_attn[:, 2 * E + lo:2 * E + hi]
        wqkv = np.concatenate([Wq, Wk, Wv], axis=1).astype(np.float32)
        xt = np.ascontiguousarray(x[b].T).astype(np.float32)
        if with_bias:
            brow = np.concatenate([
                b_attn[lo:hi] * np.float32(0.125),
                b_attn[E + lo:E + hi],
                b_attn[2 * E + lo:2 * E + hi]]).astype(np.float32)
            wqkv = np.concatenate(
                [wqkv, brow[None, :], np.zeros((P - 1, 3 * DL), np.float32)], axis=0)
            xt = np.concatenate(
                [xt, np.ones((1, T), np.float32), np.zeros((P - 1, T), np.float32)],
                axis=0)
        im = {
            "xT": np.ascontiguousarray(xt).astype(BF16),
            "wqkv": np.ascontiguousarray(wqkv).astype(BF16),
            "wproj": np.ascontiguousarray(W_proj[lo:hi, :]).astype(BF16),
        }
        if not causal:
            im["maskT"] = maskT_bf
        in_maps.append(im)
    return in_maps


def expected_partial(x, mask, W_attn, b_attn, W_proj, core):
    """Numpy reference for ONE core's partial output (for sim testing)."""
    b, hg = core // 2, core % 2
    lo, hi = hg * DL, (hg + 1) * DL
    q = x[b] @ W_attn[:, lo:hi] + b_attn[lo:hi]
    k = x[b] @ W_attn[:, E + lo:E + hi] + b_attn[E + lo:E + hi]
    v = x[b] @ W_attn[:, 2 * E + lo:2 * E + hi] + b_attn[2 * E + lo:2 * E + hi]
    q = q.reshape(T, HL, D)
    k = k.reshape(T, HL, D)
    v = v.reshape(T, HL, D)
    att = np.einsum('qhd,khd->hqk', q, k) / np.sqrt(D)
    m = np.asarray(mask).reshape(T, T)
    att = np.where(m[None] == 0, np.float32(-1e20), att)
    att = att - att.max(axis=-1, keepdims=True)
    att = np.exp(att)
    att = att / att.sum(axis=-1, keepdims=True)
    o = np.einsum('hqk,khd->qhd', att, v).reshape(T, DL)
    return o @ W_proj[lo:hi, :]


def kernel(x, mask, W_attn, b_attn, W_proj, b_proj):
    global LAST_RESULT
    from concourse.bass_utils import run_bass_kernel_spmd

    x = np.asarray(x, dtype=np.float32)
    W_attn = np.asarray(W_attn, dtype=np.float32)
    b_attn = np.asarray(b_attn, dtype=np.float32)
    W_proj = np.asarray(W_proj, dtype=np.float32)
    b_proj = np.asarray(b_proj, dtype=np.float32)

    mask2d = np.asarray(mask).reshape(T, T)
    causal = bool(np.array_equal(mask2d != 0, np.tril(np.ones((T, T), bool))))
    if not causal and not (mask2d != 0).any(axis=1).all():
        # A fully-masked query row: reference softmax degenerates to uniform
        # attention; not representable in the 0/1-multiply fast path.  This
        # cannot occur for the causal mask; fall back to exact host math.
        y = np.stack([
            sum(expected_partial(x, mask, W_attn, b_attn, W_proj, 2 * b + hg)
                for hg in range(2))
            for b in range(B)]).astype(np.float32)
        return y + b_proj
    with_bias = bool(np.any(b_attn))

    nc = _get_graph(causal, with_bias)
    in_maps = make_in_maps(x, mask, W_attn, b_attn, W_proj, b_proj,
                           causal, with_bias)
    trace = bool(int(os.environ.get("CK_TRACE", "0")))
    res = run_bass_kernel_spmd(nc, in_maps, core_ids=list(range(NCORES)),
                               trace=trace)
    LAST_RESULT = res
    y = np.empty((B, T, E), np.float32)
    for b in range(B):
        y[b] = res.results[2 * b]["out"].astype(np.float32) \
             + res.results[2 * b + 1]["out"].astype(np.float32)
    return y + b_proj


# revision 12
# speedup vs baseline: 1.4241x; 1.0887x over previous
"""Trainium2 (8 NeuronCores) kernel for a GPT-2 style causal attention block.

Reference math (per batch b):
    qkv = x @ W_attn + b_attn            # [T, 3E]
    q,k,v split -> heads H=16, D=64
    att = softmax(mask(q k^T / sqrt(D))) # causal mask
    y   = (att @ v) @ W_proj + b_proj    # [T, E]

Sharding (8 cores, no collectives):
    core c = (batch b = c//2, head-group hg = c%2 of 8 heads).
    Each core computes a PARTIAL y[b] = O_local @ W_proj[rows of its heads].
    Host sums the two partials per batch and adds b_proj (exact, commutes).

Device kernel per core (all bf16 matmuls, fp32 PSUM accumulate):
    phase 1: Q^T, K^T (feats on partitions) and V (rows on partitions) via
             matmuls from host-fed x^T and W shards.  1/sqrt(D) is folded
             into the Q columns of W on the host (exact: /8 is a pow2).
    phase 2: per (head, q-chunk of 512): S^T tiles [128 k, 512 q] on PE,
             exp on ACT (no max-subtraction needed: scores are O(1) by
             construction), causal masking by memset + multiply with a
             128x128 triangular tile, O'^T accumulation with V' that has a
             ones-column appended -> row 64 of O' is the softmax denominator.
             Normalization is fused into the PSUM->SBUF copy (broadcast the
             denominator row via DMA, reciprocal, multiply).
    phase 3: y_partial = O @ W_proj_shard, PSUM -> SBUF -> DRAM (f32).
"""

import os
import numpy as np
import ml_dtypes

B, T, E, H = 4, 2048, 1024, 16
D = E // H            # 64
NCORES = 8
HL = H // 2           # local heads per core
DL = HL * D           # 512 local attention feats
QC = 512              # q-chunk width
NQC = T // QC         # 4
NKT = T // 128        # 16 k-tiles
P = 128

BF16 = ml_dtypes.bfloat16

_graph_cache = {}
LAST_RESULT = None    # BassKernelResults of the most recent run (for test.py)


def _build(causal: bool, with_bias: bool):
    import concourse.bass as bass  # noqa: F401
    import concourse.tile as tile
    from concourse import bacc, mybir
    from concourse.masks import make_upper_triangular

    bf16 = mybir.dt.bfloat16
    f32 = mybir.dt.float32
    Exp = mybir.ActivationFunctionType.Exp

    KIN = 1152 if with_bias else 1024   # qkv contraction (pad bias row to a full tile)
    NKIN = KIN // P

    nc = bacc.Bacc("TRN2", target_bir_lowering=False, debug=False,
                   num_devices=NCORES)
    xT = nc.declare_dram_parameter("xT", [KIN, T], bf16, isOutput=False)
    wqkv = nc.declare_dram_parameter("wqkv", [KIN, 3 * DL], bf16, isOutput=False)
    wproj = nc.declare_dram_parameter("wproj", [DL, E], bf16, isOutput=False)
    if not causal:
        maskT = nc.declare_dram_parameter("maskT", [T, T], bf16, isOutput=False)
    out = nc.declare_dram_parameter("out", [T, E], f32, isOutput=True)

    with tile.TileContext(nc) as tc, \
         tc.tile_pool(name="persist", bufs=1) as persist:
        # ---- persistent SBUF tensors ----
        xT_sb = persist.tile([P, NKIN, T], bf16, tag="xT_sb", name="xT_sb")
        wq_sb = persist.tile([P, NKIN, 3 * DL], bf16, tag="wq_sb", name="wq_sb")
        wp_sb = persist.tile([P, 4, E], bf16, tag="wp_sb", name="wp_sb")
        qT_sb = persist.tile([P, 4, T], bf16, tag="qT_sb", name="qT_sb")
        kT_sb = persist.tile([P, 4, T], bf16, tag="kT_sb", name="kT_sb")
        vP_sb = persist.tile([P, NKT, HL, D + 1], bf16, tag="vP_sb", name="vP_sb")
        oT_sb = persist.tile([P, 4, T], bf16, tag="oT_sb", name="oT_sb")
        band = persist.tile([P, P], bf16, tag="band", name="band")

        for kt in range(NKIN):
            nc.sync.dma_start(out=xT_sb[:, kt, :], in_=xT[kt * P:(kt + 1) * P, :])
            nc.sync.dma_start(out=wq_sb[:, kt, 2 * DL:3 * DL],
                              in_=wqkv[kt * P:(kt + 1) * P, 2 * DL:3 * DL])
        for kt in range(NKIN):
            nc.sync.dma_start(out=wq_sb[:, kt, 0:2 * DL],
                              in_=wqkv[kt * P:(kt + 1) * P, 0:2 * DL])
        for g in range(4):
            nc.sync.dma_start(out=wp_sb[:, g, :], in_=wproj[g * P:(g + 1) * P, :])
        if causal:
            # band[kp, qf] = 1.0 where kp <= qf else 0  (keep k <= q)
            make_upper_triangular(nc, band[:, :], val=1.0, diag=True)
        nc.vector.memset(vP_sb[:, :, :, D:D + 1], 1.0)

        with (
            tc.tile_pool(name="psA", bufs=2, space="PSUM") as psA,
            tc.tile_pool(name="psS", bufs=2, space="PSUM") as psS,
            tc.tile_pool(name="psO", bufs=2, space="PSUM") as psO,
            tc.tile_pool(name="sbw", bufs=3) as sbw,
            tc.tile_pool(name="sbm", bufs=2) as sbm,
            tc.tile_pool(name="drp", bufs=2, space="DRAM") as drp,
        ):
            # ---- phase 1a: V = x @ Wv  (rows on partitions) ----
            for rt in range(NKT):
                ps_v = psA.tile([P, DL], f32, tag="mm512", name="ps_v")
                for kt in range(NKIN):
                    nc.tensor.matmul(
                        ps_v[:],
                        lhsT=xT_sb[:, kt, rt * P:(rt + 1) * P],
                        rhs=wq_sb[:, kt, 2 * DL:3 * DL],
                        start=(kt == 0), stop=(kt == NKIN - 1))
                nc.vector.tensor_copy(
                    vP_sb[:, rt, :, 0:D],
                    ps_v[:].rearrange("p (h d) -> p h d", h=HL))

            def emit_qk(g):
                # ---- phase 1b: Q^T, K^T for head-pair g ----
                for rc in range(NQC):
                    ps_q = psA.tile([P, QC], f32, tag="mm512", name="ps_q")
                    for kt in range(NKIN):
                        nc.tensor.matmul(
                            ps_q[:],
                            lhsT=wq_sb[:, kt, g * P:(g + 1) * P],
                            rhs=xT_sb[:, kt, rc * QC:(rc + 1) * QC],
                            start=(kt == 0), stop=(kt == NKIN - 1))
                    nc.vector.tensor_copy(qT_sb[:, g, rc * QC:(rc + 1) * QC], ps_q[:])
                    ps_k = psA.tile([P, QC], f32, tag="mm512", name="ps_k")
                    for kt in range(NKIN):
                        nc.tensor.matmul(
                            ps_k[:],
                            lhsT=wq_sb[:, kt, DL + g * P:DL + (g + 1) * P],
                            rhs=xT_sb[:, kt, rc * QC:(rc + 1) * QC],
                            start=(kt == 0), stop=(kt == NKIN - 1))
                    nc.vector.tensor_copy(kT_sb[:, g, rc * QC:(rc + 1) * QC], ps_k[:])

            def emit_attn(g):
                # ---- phase 2: attention for heads 2g, 2g+1, interleaved ----
                for qc in range(NQC):
                    nkt = 4 * (qc + 1) if causal else NKT
                    ps_o = [psO.tile([P, QC], f32, tag="ps_o", name=f"ps_o{j}") for j in range(2)]
                    for kt2 in range(nkt // 2):
                        pTs = []
                        # live-column start per slab (diagonal tiles are
                        # fully masked below column kt*128 - qc*512)
                        ss = [max(0, (2 * kt2 + t2) * P - qc * QC) if causal else 0
                              for t2 in range(2)]
                        for j in range(2):
                            ps_s = psS.tile([P, 2 * QC], f32, tag="ps_s", name=f"ps_s{j}")
                            for t2 in range(2):
                                kt = 2 * kt2 + t2
                                nc.tensor.matmul(
                                    ps_s[:, t2 * QC + ss[t2]:(t2 + 1) * QC],
                                    lhsT=kT_sb[j * D:(j + 1) * D, g, kt * P:(kt + 1) * P],
                                    rhs=qT_sb[j * D:(j + 1) * D, g,
                                              qc * QC + ss[t2]:(qc + 1) * QC],
                                    start=True, stop=True)
                            pT = sbw.tile([P, 2 * QC], bf16, tag=f"pT{j}", name=f"pT{j}")
                            pTs.append(pT)
                            if ss[0] == 0 and ss[1] == 0:
                                nc.scalar.activation(out=pT[:], in_=ps_s[:], func=Exp)
                            else:
                                nc.scalar.activation(out=pT[:, ss[0]:QC],
                                                     in_=ps_s[:, ss[0]:QC], func=Exp)
                                nc.scalar.activation(out=pT[:, QC + ss[1]:],
                                                     in_=ps_s[:, QC + ss[1]:], func=Exp)
                            if causal:
                                for t2 in range(2):
                                    kt = 2 * kt2 + t2
                                    if kt >= 4 * qc:  # diagonal-band k-tile
                                        s = ss[t2]
                                        nc.vector.tensor_mul(
                                            pT[:, t2 * QC + s:t2 * QC + s + P],
                                            pT[:, t2 * QC + s:t2 * QC + s + P],
                                            band[:, :])
                            else:
                                msk = sbm.tile([P, 2 * QC], bf16, tag="msk", name="msk")
                                for t2 in range(2):
                                    kt = 2 * kt2 + t2
                                    nc.sync.dma_start(
                                        out=msk[:, t2 * QC:(t2 + 1) * QC],
                                        in_=maskT[kt * P:(kt + 1) * P, qc * QC:(qc + 1) * QC])
                                nc.vector.tensor_mul(pT[:], pT[:], msk[:])
                        for j in range(2):
                            for t2 in range(2):
                                kt = 2 * kt2 + t2
                                nc.tensor.matmul(
                                    ps_o[j][0:D + 1, ss[t2]:],
                                    lhsT=vP_sb[:, kt, 2 * g + j, :],
                                    rhs=pTs[j][:, t2 * QC + ss[t2]:(t2 + 1) * QC],
                                    start=(kt == 0), stop=(kt == nkt - 1))
                    for j in range(2):
                        # early-release ps_o: copy O and rowsum to SBUF, then
                        # normalize off-PSUM:  O[d, q] / rowsum[q]
                        oU = sbm.tile([D, QC], bf16, tag="oU", name="oU")
                        nc.vector.tensor_copy(oU[:], ps_o[j][0:D, :])
                        rrow = sbm.tile([1, QC], f32, tag="rrow", name="rrow")
                        nc.vector.tensor_copy(rrow[:], ps_o[j][D:D + 1, :])
                        rdr = drp.tile([1, QC], f32, tag="rdr", name="rdr")
                        nc.sync.dma_start(out=rdr[:], in_=rrow[:])
                        rb = sbm.tile([D, QC], f32, tag="rb", name="rb")
                        nc.sync.dma_start(out=rb[:], in_=rdr[:].to_broadcast((D, QC)))
                        nc.vector.reciprocal_approx_fast(out=rb[:], in_=rb[:])
                        nc.vector.tensor_mul(
                            oT_sb[j * D:(j + 1) * D, g, qc * QC:(qc + 1) * QC],
                            oU[:], rb[:])

            # software-pipeline the emission: QK of pair g+1 ahead of attn(g)
            emit_qk(0)
            for g in range(4):
                if g + 1 < 4:
                    emit_qk(g + 1)
                emit_attn(g)

            # ---- phase 3: y_partial = O @ W_proj_shard ----
            for rt in range(NKT):
                for nb in range(2):
                    ps_y = psA.tile([P, 512], f32, tag="mm512", name="ps_y")
                    for g in range(4):
                        nc.tensor.matmul(
                            ps_y[:],
                            lhsT=oT_sb[:, g, rt * P:(rt + 1) * P],
                            rhs=wp_sb[:, g, nb * 512:(nb + 1) * 512],
                            start=(g == 0), stop=(g == 3))
                    y_sb = sbw.tile([P, 512], f32, tag="y_sb", name="y_sb")
                    nc.vector.tensor_copy(y_sb[:], ps_y[:])
                    nc.sync.dma_start(
                        out=out[rt * P:(rt + 1) * P, nb * 512:(nb + 1) * 512],
                        in_=y_sb[:])

    nc.compile()
    return nc


def _get_graph(causal: bool, with_bias: bool):
    key = (causal, with_bias)
    if key not in _graph_cache:
        _graph_cache[key] = _build(causal, with_bias)
    return _graph_cache[key]


def make_in_maps(x, mask, W_attn, b_attn, W_proj, b_proj, causal, with_bias):
    """Host-side sharding: per-core input dict (bf16)."""
    in_maps = []
    maskT_bf = None
    if not causal:
        m = np.asarray(mask).reshape(T, T)
        maskT_bf = np.ascontiguousarray(m.T).astype(BF16)
    for c in range(NCORES):
        b, hg = c // 2, c % 2
        lo, hi = hg * DL, (hg + 1) * DL
        Wq = W_attn[:, lo:hi] * np.float32(0.125)
        Wk = W_attn[:, E + lo:E + hi]
        Wv = W_attn[:, 2 * E + lo:2 * E + hi]
        wqkv = np.concatenate([Wq, Wk, Wv], axis=1).astype(np.float32)
        xt = np.ascontiguousarray(x[b].T).astype(np.float32)
        if with_bias:
            brow = np.concatenate([
                b_attn[lo:hi] * np.float32(0.125),
                b_attn[E + lo:E + hi],
                b_attn[2 * E + lo:2 * E + hi]]).astype(np.float32)
            wqkv = np.concatenate(
                [wqkv, brow[None, :], np.zeros((P - 1, 3 * DL), np.float32)], axis=0)
            xt = np.concatenate(
                [xt, np.ones((1, T), np.float32), np.zeros((P - 1, T), np.float32)],
                axis=0)
        im = {
            "xT": np.ascontiguousarray(xt).astype(BF16),
            "wqkv": np.ascontiguousarray(wqkv).astype(BF16),
            "wproj": np.ascontiguousarray(W_proj[lo:hi, :]).astype(BF16),
        }
        if not causal:
            im["maskT"] = maskT_bf
        in_maps.append(im)
    return in_maps


def expected_partial(x, mask, W_attn, b_attn, W_proj, core):
    """Numpy reference for ONE core's partial output (for sim testing)."""
    b, hg = core // 2, core % 2
    lo, hi = hg * DL, (hg + 1) * DL
    q = x[b] @ W_attn[:, lo:hi] + b_attn[lo:hi]
    k = x[b] @ W_attn[:, E + lo:E + hi] + b_attn[E + lo:E + hi]
    v = x[b] @ W_attn[:, 2 * E + lo:2 * E + hi] + b_attn[2 * E + lo:2 * E + hi]
    q = q.reshape(T, HL, D)
    k = k.reshape(T, HL, D)
    v = v.reshape(T, HL, D)
    att = np.einsum('qhd,khd->hqk', q, k) / np.sqrt(D)
    m = np.asarray(mask).reshape(T, T)
    att = np.where(m[None] == 0, np.float32(-1e20), att)
    att = att - att.max(axis=-1, keepdims=True)
    att = np.exp(att)
    att = att / att.sum(axis=-1, keepdims=True)
    o = np.einsum('hqk,khd->qhd', att, v).reshape(T, DL)
    return o @ W_proj[lo:hi, :]


def kernel(x, mask, W_attn, b_attn, W_proj, b_proj):
    global LAST_RESULT
    from concourse.bass_utils import run_bass_kernel_spmd

    x = np.asarray(x, dtype=np.float32)
    W_attn = np.asarray(W_attn, dtype=np.float32)
    b_attn = np.asarray(b_attn, dtype=np.float32)
    W_proj = np.asarray(W_proj, dtype=np.float32)
    b_proj = np.asarray(b_proj, dtype=np.float32)

    mask2d = np.asarray(mask).reshape(T, T)
    causal = bool(np.array_equal(mask2d != 0, np.tril(np.ones((T, T), bool))))
    if not causal and not (mask2d != 0).any(axis=1).all():
        # A fully-masked query row: reference softmax degenerates to uniform
        # attention; not representable in the 0/1-multiply fast path.  This
        # cannot occur for the causal mask; fall back to exact host math.
        y = np.stack([
            sum(expected_partial(x, mask, W_attn, b_attn, W_proj, 2 * b + hg)
                for hg in range(2))
            for b in range(B)]).astype(np.float32)
        return y + b_proj
    with_bias = bool(np.any(b_attn))

    nc = _get_graph(causal, with_bias)
    in_maps = make_in_maps(x, mask, W_attn, b_attn, W_proj, b_proj,
                           causal, with_bias)
    trace = bool(int(os.environ.get("CK_TRACE", "0")))
    res = run_bass_kernel_spmd(nc, in_maps, core_ids=list(range(NCORES)),
                               trace=trace)
    LAST_RESULT = res
    y = np.empty((B, T, E), np.float32)
    for b in range(B):
        y[b] = res.results[2 * b]["out"].astype(np.float32) \
             + res.results[2 * b + 1]["out"].astype(np.float32)
    return y + b_proj
